# revision 5
# baseline (speedup 1.0000x reference)
"""Chamfer loss (single-direction) Trainium2 Bass kernel.

Problem: pc_src [B=4, 3, M=8192], pc_dst [B=4, 3, N=8192] (fp32).
  d2[b,m,n] = ||src[b,:,m] - dst[b,:,n]||^2
  out = mean over (b,m) of sqrt(min_n d2[b,m,n])

Sharding: 8 cores = 4 batches x 2 M-halves. Each core handles one batch's
dst [3, 8192] and a 4096-point slice of that batch's src. The min over n is
complete per core; the host concatenates per-core min-d2 vectors and does
the (tiny, O(B*M)) sqrt + mean.

Device algorithm per core — fp8e5m2 multi-level augmented matmul in
DoubleRow perf mode (0.5 PE cycles per output element, 2x over bf16):
  Each fp32 value is decomposed into NLEV=6 e5m2 levels x = sum_a x_a with
  |x_a| shrinking ~8x per level, so the reconstruction residual is ~2^-18
  relative. e5m2's wide exponent range (subnormals to 2^-16) keeps every
  level representable — e4m3 would hit its subnormal floor at level 2.
  fp8 x fp8 products are exact in the fp32 PSUM accumulator.

  d2 = ||s||^2 + ||d||^2 - 2 s.d expands over level pairs; all pairs with
  a + b <= LMAX=6 are kept (dropped terms ~8^-7):
    cross rows:  lhsT = -2*s_a[coord],  rhs = d_b[coord]   (26 pairs x 3)
    norm rows:   lhsT = ssq_a, rhs = 1  /  lhsT = 1, rhs = dsq_b  (6 + 6)
  = 90 contraction rows, packed as [45, 2, *] for DoubleRow, which
  contracts over (k, i): out[m,n] = sum_k sum_i lhsT[k,i,m]*rhs[k,i,n].
  K is free on the PE; only the moving/output element count is charged.

  The min-reduce runs on the VectorEngine with stock tensor_tensor_reduce,
  one instruction per pair of [128, 1024] PSUM tiles:
    mins4[:, mt, pr] = min(BIG, min_free(min(psumA, sbufB)))
  (2 distance elements per cycle per lane, both read ports). ScalarE
  (otherwise idle) stages psumB into SBUF — the ISA allows only one
  non-scalar DVE input in PSUM. The 4 pair results per M-tile land in
  independent accum slots (no serial chain), and one final grouped
  tensor_reduce collapses mins4 [P, m_tiles, 4] -> [P, m_tiles].
"""

import ml_dtypes
import numpy as np

import concourse.bass as bass
import concourse.mybir as mybir
from concourse import bacc
from concourse import dve_ops as _dve_ops
from concourse.bass_utils import run_bass_kernel_spmd
from concourse.dve_spec import AluOp, C0, Spec, Src0, Src1, lower, minn
from concourse.dve_uop import DveOpSpec
from concourse.tile import TileContext

F32 = mybir.dt.float32
FP8 = mybir.dt.float8e5
BIG = 3.0e38
NP_FP8 = ml_dtypes.float8_e5m2


def _make_min2_op():
    """Register a custom DVE op: out = min(in0, in1); accum_out = min(s0, min_k out).

    Stock tensor_tensor_reduce has no ucode behind it on this target (runtime
    INTERNAL error), so the pair-min + free-dim-min-reduce is a custom op.
    """
    name = "MIN2_REDUCE_ANT"
    for existing in _dve_ops.OPS:
        if existing.name == name:
            return existing
    spec = Spec(
        body=minn(Src0, Src1),
        accum=AluOp.MIN,
        accum_init=C0,
        reference=lambda in0, in1, c0, c1, c2: (
            np.minimum(in0, in1),
            np.minimum(
                np.asarray(c0, np.float32).reshape(-1, 1)
                if isinstance(c0, np.ndarray)
                else np.float32(c0),
                np.minimum(in0, in1).min(axis=-1, keepdims=True),
            )
            * np.ones((in0.shape[0], 1), np.float32),
        ),
    )
    opcode = _dve_ops._CUSTOM_DVE_ROW_BASE + len(_dve_ops.OPS)
    shas = {}
    for ver in ("v3", "v4"):
        try:
            tmp = DveOpSpec(
                name=name,
                opcode=opcode,
                uops=lower(spec, ver=ver),
                rd1_en=_dve_ops.has_src1(spec),
            )
            shas[ver] = tmp.sha(ver)
        except Exception:
            pass
    op = _dve_ops.DveOp(name, spec, subdim=False, uops_sha=shas)
    _dve_ops.OPS.append(op)
    _dve_ops.CUSTOM_DVE_SPECS[name] = spec
    _dve_ops._SUB_OPCODE_FOR_NAME[name] = opcode
    return op


MIN2 = _make_min2_op()

# Problem constants (hardcoded per contract)
B = 4
D = 3
M = 8192
N = 8192
N_CORES = 8
M_SHARD = M // 2  # 4096 src points per core

NLEV = 6         # e5m2 split levels per value
LMAX = 6         # keep cross pairs with a+b <= LMAX
_PAIRS = [(a, b) for a in range(NLEV) for b in range(NLEV) if a + b <= LMAX]
K_ROWS = 3 * len(_PAIRS) + 2 * NLEV  # 90
K_HALF = (K_ROWS + 1) // 2           # 45 physical partitions (DoubleRow)

P = 128          # output partitions per M-tile
MM_N = 512       # matmul output free dim (fp32 PSUM: 1 bank)
PSUM_FD = 1024   # min-reduce operand width (2 PSUM banks)


def build_nc(m_shard: int = M_SHARD, n: int = N, reps: int = 1) -> bass.Bass:
    assert m_shard % P == 0 and n % (4 * PSUM_FD) == 0
    m_tiles = m_shard // P
    pairs = n // (2 * PSUM_FD)  # min-reduce pairs per M-tile

    nc = bacc.Bacc()
    src = nc.dram_tensor("src", [K_HALF, 2, m_shard], FP8, kind="ExternalInput")
    dst = nc.dram_tensor("dst", [K_HALF, 2, n], FP8, kind="ExternalInput")
    out = nc.dram_tensor("out", [P, m_tiles], F32, kind="ExternalOutput")

    with TileContext(nc) as tc:
        with (
            tc.tile_pool(name="big", bufs=1) as big,
            tc.tile_pool(name="scr", bufs=3) as scr,
            tc.tile_pool(name="psum", bufs=4, space="PSUM") as psum,
        ):
            srcT = big.tile([K_HALF, 2, m_shard], FP8)
            dstT = big.tile([K_HALF, 2, n], FP8)
            mins4 = big.tile([P, m_tiles, 4], F32)
            mins = big.tile([P, m_tiles], F32)

            nc.sync.dma_start(out=srcT, in_=src[:, :, :])
            nc.sync.dma_start(out=dstT, in_=dst[:, :, :])

            # --- main loop: 1 M-tile = 128 src points vs all n dst points -
            for mt in [t for _ in range(reps) for t in range(m_tiles)]:
                lhsT = srcT[:, :, mt * P : (mt + 1) * P]  # [45, 2, 128]
                for pr in range(pairs):
                    base = pr * 2 * PSUM_FD
                    pA = psum.tile([P, PSUM_FD], F32, tag="ps")
                    pB = psum.tile([P, PSUM_FD], F32, tag="ps")
                    for t, pt in ((0, pA), (1, pB)):
                        for h in range(PSUM_FD // MM_N):
                            n0 = base + t * PSUM_FD + h * MM_N
                            nc.tensor.matmul(
                                pt[:, h * MM_N : (h + 1) * MM_N],
                                lhsT,
                                dstT[:, :, n0 : n0 + MM_N],
                                start=True,
                                stop=True,
                                perf_mode=mybir.MatmulPerfMode.DoubleRow,
                            )
                    # ISA: only one non-scalar input may live in PSUM, so the
                    # (otherwise idle) ScalarE stages pB into SBUF first.
                    sB = scr.tile([P, PSUM_FD], F32, tag="cp")
                    nc.scalar.copy(out=sB, in_=pB)
                    ttr_out = scr.tile([P, PSUM_FD], F32, tag="ttr")
                    nc.vector._custom_dve(
                        MIN2,
                        out=ttr_out,
                        in0=pA,
                        in1=sB,
                        s0=BIG,
                        accum_out=mins4[:, mt, pr : pr + 1],
                    )

            nc.vector.tensor_reduce(
                out=mins[:, :],
                in_=mins4[:, :, :],
                op=mybir.AluOpType.min,
                axis=mybir.AxisListType.X,
            )
            nc.sync.dma_start(out=out[:, :], in_=mins[:, :])

    nc.finalize()
    return nc


def _split_levels(x64: np.ndarray, nlev: int = NLEV) -> list[np.ndarray]:
    """Decompose float64 x into nlev fp8e5m2 levels, x ~= sum(levels)."""
    levels = []
    r = x64.copy()
    for _ in range(nlev):
        li = r.astype(np.float32).astype(NP_FP8)
        levels.append(li)
        r = r - li.astype(np.float64)
    return levels


def _prep_operands(src_f32: np.ndarray, dst_f32: np.ndarray) -> tuple[np.ndarray, np.ndarray]:
    """Build the [45, 2, m] stationary and [45, 2, n] moving fp8 operands."""
    m = src_f32.shape[1]
    n = dst_f32.shape[1]
    s64 = src_f32.astype(np.float64)
    d64 = dst_f32.astype(np.float64)
    s_lev = _split_levels(s64)                      # each [3, m]
    d_lev = _split_levels(d64)                      # each [3, n]
    ssq = _split_levels(np.sum(s64 * s64, axis=0))  # each [m]
    dsq = _split_levels(np.sum(d64 * d64, axis=0))  # each [n]

    lhsT = np.zeros((2 * K_HALF, m), NP_FP8)
    rhs = np.zeros((2 * K_HALF, n), NP_FP8)
    r = 0
    for a, b in _PAIRS:
        neg2sa = (-2.0 * s_lev[a].astype(np.float64)).astype(NP_FP8)  # exact *2
        lhsT[r : r + 3] = neg2sa
        rhs[r : r + 3] = d_lev[b]
        r += 3
    for a in range(NLEV):
        lhsT[r] = ssq[a]
        rhs[r] = NP_FP8(1.0)
        r += 1
    for b in range(NLEV):
        lhsT[r] = NP_FP8(1.0)
        rhs[r] = dsq[b]
        r += 1
    assert r == K_ROWS
    return lhsT.reshape(K_HALF, 2, m), rhs.reshape(K_HALF, 2, n)


_NC_CACHE: dict = {}


def _get_nc(m_shard: int, n: int) -> bass.Bass:
    key = (m_shard, n)
    if key not in _NC_CACHE:
        _NC_CACHE[key] = build_nc(m_shard, n)
    return _NC_CACHE[key]


LAST_RESULTS = None  # test harness can inspect exec_time_ns etc.


def kernel(pc_src: np.ndarray, pc_dst: np.ndarray) -> np.ndarray:
    pc_src = np.ascontiguousarray(np.asarray(pc_src), dtype=np.float32)
    pc_dst = np.ascontiguousarray(np.asarray(pc_dst), dtype=np.float32)
    assert pc_src.shape == (B, D, M) and pc_dst.shape == (B, D, N)

    nc = _get_nc(M_SHARD, N)

    in_maps = []
    for c in range(N_CORES):
        b, h = divmod(c, 2)
        lhsT, rhs = _prep_operands(
            pc_src[b, :, h * M_SHARD : (h + 1) * M_SHARD], pc_dst[b]
        )
        in_maps.append({"src": lhsT, "dst": rhs})

    global LAST_RESULTS
    LAST_RESULTS = run_bass_kernel_spmd(nc, in_maps, core_ids=list(range(N_CORES)))

    # host: O(B*M) postprocess (sqrt + mean) over per-core min-d2 columns
    md2 = np.concatenate(
        [LAST_RESULTS.results[c]["out"].T.reshape(-1) for c in range(N_CORES)]
    )
    md2 = np.maximum(md2, 0.0)
    dists = np.sqrt(md2, dtype=np.float32)
    return np.asarray(np.mean(dists, dtype=np.float32), dtype=np.float32)


# revision 6
# speedup vs baseline: 1.2300x; 1.2300x over previous
"""Chamfer loss (single-direction) Trainium2 Bass kernel.

Problem: pc_src [B=4, 3, M=8192], pc_dst [B=4, 3, N=8192] (fp32).
  d2[b,m,n] = ||src[b,:,m] - dst[b,:,n]||^2
  out = mean over (b,m) of sqrt(min_n d2[b,m,n])

Sharding: 8 cores = 4 batches x 2 M-halves. Each core handles one batch's
dst [3, 8192] and a 4096-point slice of that batch's src. The min over n is
complete per core; the host concatenates per-core min-d2 vectors and does
the (tiny, O(B*M)) sqrt + mean.

Device algorithm per core — fp8e5m2 multi-level augmented matmul in
DoubleRow perf mode (0.5 PE cycles per output element, 2x over bf16):
  Each fp32 value is decomposed into NLEV=6 e5m2 levels x = sum_a x_a with
  |x_a| shrinking ~8x per level, so the reconstruction residual is ~2^-18
  relative. e5m2's wide exponent range (subnormals to 2^-16) keeps every
  level representable — e4m3 would hit its subnormal floor at level 2.
  fp8 x fp8 products are exact in the fp32 PSUM accumulator.

  d2 = ||s||^2 + ||d||^2 - 2 s.d expands over level pairs; all pairs with
  a + b <= LMAX=6 are kept (dropped terms ~8^-7):
    cross rows:  lhsT = -2*s_a[coord],  rhs = d_b[coord]   (26 pairs x 3)
    norm rows:   lhsT = ssq_a, rhs = 1  /  lhsT = 1, rhs = dsq_b  (6 + 6)
  = 90 contraction rows, packed as [45, 2, *] for DoubleRow, which
  contracts over (k, i): out[m,n] = sum_k sum_i lhsT[k,i,m]*rhs[k,i,n].
  K is free on the PE; only the moving/output element count is charged.

  The min-reduce runs on the VectorEngine with stock tensor_tensor_reduce,
  one instruction per pair of [128, 1024] PSUM tiles:
    mins4[:, mt, pr] = min(BIG, min_free(min(psumA, sbufB)))
  (2 distance elements per cycle per lane, both read ports). ScalarE
  (otherwise idle) stages psumB into SBUF — the ISA allows only one
  non-scalar DVE input in PSUM. The 4 pair results per M-tile land in
  independent accum slots (no serial chain), and one final grouped
  tensor_reduce collapses mins4 [P, m_tiles, 4] -> [P, m_tiles].
"""

import ml_dtypes
import numpy as np

import concourse.bass as bass
import concourse.mybir as mybir
from concourse import bacc
from concourse import dve_ops as _dve_ops
from concourse.bass_utils import run_bass_kernel_spmd
from concourse.dve_spec import AluOp, C0, Spec, Src0, Src1, lower, minn
from concourse.dve_uop import DveOpSpec
from concourse.tile import TileContext

F32 = mybir.dt.float32
FP8 = mybir.dt.float8e5
BIG = 3.0e38
NP_FP8 = ml_dtypes.float8_e5m2


def _make_min2_op():
    """Register a custom DVE op: out = min(in0, in1); accum_out = min(s0, min_k out).

    Stock tensor_tensor_reduce has no ucode behind it on this target (runtime
    INTERNAL error), so the pair-min + free-dim-min-reduce is a custom op.
    """
    name = "MIN2_REDUCE_ANT"
    for existing in _dve_ops.OPS:
        if existing.name == name:
            return existing
    spec = Spec(
        body=minn(Src0, Src1),
        accum=AluOp.MIN,
        accum_init=C0,
        reference=lambda in0, in1, c0, c1, c2: (
            np.minimum(in0, in1),
            np.minimum(
                np.asarray(c0, np.float32).reshape(-1, 1)
                if isinstance(c0, np.ndarray)
                else np.float32(c0),
                np.minimum(in0, in1).min(axis=-1, keepdims=True),
            )
            * np.ones((in0.shape[0], 1), np.float32),
        ),
    )
    opcode = _dve_ops._CUSTOM_DVE_ROW_BASE + len(_dve_ops.OPS)
    shas = {}
    for ver in ("v3", "v4"):
        try:
            tmp = DveOpSpec(
                name=name,
                opcode=opcode,
                uops=lower(spec, ver=ver),
                rd1_en=_dve_ops.has_src1(spec),
            )
            shas[ver] = tmp.sha(ver)
        except Exception:
            pass
    op = _dve_ops.DveOp(name, spec, subdim=False, uops_sha=shas)
    _dve_ops.OPS.append(op)
    _dve_ops.CUSTOM_DVE_SPECS[name] = spec
    _dve_ops._SUB_OPCODE_FOR_NAME[name] = opcode
    return op


MIN2 = _make_min2_op()

# Problem constants (hardcoded per contract)
B = 4
D = 3
M = 8192
N = 8192
N_CORES = 8
M_SHARD = M // 2  # 4096 src points per core

NLEV = 6         # e5m2 split levels per value
LMAX = 6         # keep cross pairs with a+b <= LMAX
_PAIRS = [(a, b) for a in range(NLEV) for b in range(NLEV) if a + b <= LMAX]
K_ROWS = 3 * len(_PAIRS) + 2 * NLEV  # 90
K_HALF = (K_ROWS + 1) // 2           # 45 physical partitions (DoubleRow)

P = 128          # output partitions per M-tile
MM_N = 512       # matmul output free dim (fp32 PSUM: 1 bank)
PSUM_FD = 1024   # min-reduce operand width (2 PSUM banks)


def build_nc(m_shard: int = M_SHARD, n: int = N, reps: int = 1) -> bass.Bass:
    assert m_shard % P == 0 and n % (4 * PSUM_FD) == 0
    m_tiles = m_shard // P
    pairs = n // (2 * PSUM_FD)  # min-reduce pairs per M-tile

    nc = bacc.Bacc()
    src = nc.dram_tensor("src", [K_HALF, 2, m_shard], FP8, kind="ExternalInput")
    dst = nc.dram_tensor("dst", [K_HALF, 2, n], FP8, kind="ExternalInput")
    out = nc.dram_tensor("out", [P, m_tiles], F32, kind="ExternalOutput")

    with TileContext(nc) as tc:
        with (
            tc.tile_pool(name="big", bufs=1) as big,
            tc.tile_pool(name="scr", bufs=3) as scr,
            tc.tile_pool(name="psum", bufs=4, space="PSUM") as psum,
        ):
            srcT = big.tile([K_HALF, 2, m_shard], FP8)
            dstT = big.tile([K_HALF, 2, n], FP8)
            mins4 = big.tile([P, m_tiles, 4], F32)
            mins = big.tile([P, m_tiles], F32)

            nc.sync.dma_start(out=srcT, in_=src[:, :, :])
            nc.sync.dma_start(out=dstT, in_=dst[:, :, :])

            # --- main loop: 1 M-tile = 128 src points vs all n dst points -
            for mt in [t for _ in range(reps) for t in range(m_tiles)]:
                lhsT = srcT[:, :, mt * P : (mt + 1) * P]  # [45, 2, 128]
                for pr in range(pairs):
                    base = pr * 2 * PSUM_FD
                    pA = psum.tile([P, PSUM_FD], F32, tag="ps")
                    pB = psum.tile([P, PSUM_FD], F32, tag="ps")
                    # Fill pB FIRST: the ScalarE staging copy is on the
                    # critical PSUM-recycle chain, so it must start as early
                    # as possible; pA's matmuls then overlap the copy.
                    for t, pt in ((1, pB), (0, pA)):
                        for h in range(PSUM_FD // MM_N):
                            n0 = base + t * PSUM_FD + h * MM_N
                            nc.tensor.matmul(
                                pt[:, h * MM_N : (h + 1) * MM_N],
                                lhsT,
                                dstT[:, :, n0 : n0 + MM_N],
                                start=True,
                                stop=True,
                                perf_mode=mybir.MatmulPerfMode.DoubleRow,
                            )
                        if t == 1:
                            # ISA: only one non-scalar DVE input may live in
                            # PSUM; the (otherwise idle) ScalarE stages pB
                            # into SBUF right behind pB's matmuls.
                            sB = scr.tile([P, PSUM_FD], F32, tag="cp")
                            nc.scalar.copy(out=sB, in_=pB)
                    ttr_out = scr.tile([P, PSUM_FD], F32, tag="ttr")
                    nc.vector._custom_dve(
                        MIN2,
                        out=ttr_out,
                        in0=pA,
                        in1=sB,
                        s0=BIG,
                        accum_out=mins4[:, mt, pr : pr + 1],
                    )

            nc.vector.tensor_reduce(
                out=mins[:, :],
                in_=mins4[:, :, :],
                op=mybir.AluOpType.min,
                axis=mybir.AxisListType.X,
            )
            nc.sync.dma_start(out=out[:, :], in_=mins[:, :])

    nc.finalize()
    return nc


def _split_levels(x64: np.ndarray, nlev: int = NLEV) -> list[np.ndarray]:
    """Decompose float64 x into nlev fp8e5m2 levels, x ~= sum(levels)."""
    levels = []
    r = x64.copy()
    for _ in range(nlev):
        li = r.astype(np.float32).astype(NP_FP8)
        levels.append(li)
        r = r - li.astype(np.float64)
    return levels


def _prep_operands(src_f32: np.ndarray, dst_f32: np.ndarray) -> tuple[np.ndarray, np.ndarray]:
    """Build the [45, 2, m] stationary and [45, 2, n] moving fp8 operands."""
    m = src_f32.shape[1]
    n = dst_f32.shape[1]
    s64 = src_f32.astype(np.float64)
    d64 = dst_f32.astype(np.float64)
    s_lev = _split_levels(s64)                      # each [3, m]
    d_lev = _split_levels(d64)                      # each [3, n]
    ssq = _split_levels(np.sum(s64 * s64, axis=0))  # each [m]
    dsq = _split_levels(np.sum(d64 * d64, axis=0))  # each [n]

    lhsT = np.zeros((2 * K_HALF, m), NP_FP8)
    rhs = np.zeros((2 * K_HALF, n), NP_FP8)
    r = 0
    for a, b in _PAIRS:
        neg2sa = (-2.0 * s_lev[a].astype(np.float64)).astype(NP_FP8)  # exact *2
        lhsT[r : r + 3] = neg2sa
        rhs[r : r + 3] = d_lev[b]
        r += 3
    for a in range(NLEV):
        lhsT[r] = ssq[a]
        rhs[r] = NP_FP8(1.0)
        r += 1
    for b in range(NLEV):
        lhsT[r] = NP_FP8(1.0)
        rhs[r] = dsq[b]
        r += 1
    assert r == K_ROWS
    return lhsT.reshape(K_HALF, 2, m), rhs.reshape(K_HALF, 2, n)


_NC_CACHE: dict = {}


def _get_nc(m_shard: int, n: int) -> bass.Bass:
    key = (m_shard, n)
    if key not in _NC_CACHE:
        _NC_CACHE[key] = build_nc(m_shard, n)
    return _NC_CACHE[key]


LAST_RESULTS = None  # test harness can inspect exec_time_ns etc.


def kernel(pc_src: np.ndarray, pc_dst: np.ndarray) -> np.ndarray:
    pc_src = np.ascontiguousarray(np.asarray(pc_src), dtype=np.float32)
    pc_dst = np.ascontiguousarray(np.asarray(pc_dst), dtype=np.float32)
    assert pc_src.shape == (B, D, M) and pc_dst.shape == (B, D, N)

    nc = _get_nc(M_SHARD, N)

    in_maps = []
    for c in range(N_CORES):
        b, h = divmod(c, 2)
        lhsT, rhs = _prep_operands(
            pc_src[b, :, h * M_SHARD : (h + 1) * M_SHARD], pc_dst[b]
        )
        in_maps.append({"src": lhsT, "dst": rhs})

    global LAST_RESULTS
    LAST_RESULTS = run_bass_kernel_spmd(nc, in_maps, core_ids=list(range(N_CORES)))

    # host: O(B*M) postprocess (sqrt + mean) over per-core min-d2 columns
    md2 = np.concatenate(
        [LAST_RESULTS.results[c]["out"].T.reshape(-1) for c in range(N_CORES)]
    )
    md2 = np.maximum(md2, 0.0)
    dists = np.sqrt(md2, dtype=np.float32)
    return np.asarray(np.mean(dists, dtype=np.float32), dtype=np.float32)
